# revision 54
# baseline (speedup 1.0000x reference)
"""DIN-style attention layer on 8 Trainium2 NeuronCores.

Problem: q[B,64], k[B,200,64], v[B,200,64], mask[B,200]; per-token MLP on
DIN features concat([q,k,q-k,q*k]) -> 80 -> 40 -> 1 logits, masked softmax
over T, then attn-weighted sum of v. B=2048 sharded over 8 cores.

Math refactor (host):
  info@W1 = q@(W1a+W1c) + k@(W1b-W1c) + (q*k)@W1d   with W1=[W1a;W1b;W1c;W1d]
  => h1_b = relu( Wb_eff^T kt_b + beta_b ),  Wb_eff = (W1b-W1c) + q_b*W1d
     beta_b = q_b@(W1a+W1c) + b1   (folded in as a 65th all-ones row of kt)
bf is dropped: softmax is shift-invariant. mask applied multiplicatively
(exp first, then EXM = EX * mask with fused row-sum on the DVE).

Device (per core, 256 batches = 128 pairs = 64 chunks of 2 pairs):
software-pipelined chunk loop keeping the PE dense (L2 lags one chunk,
L3 two, softmax/transpose/out-matmul tail lags ~a group), elementwise
work balanced across DVE and ACT. Outputs accumulate in SBUF, one DMA
at the end.
"""

import os
import sys

import numpy as np

for _p in ("/opt/trn_rl_repo", "/root/.axon_site/_ro/trn_rl_repo"):
    if os.path.isdir(_p) and _p not in sys.path:
        sys.path.insert(0, _p)

import ml_dtypes

BF16 = ml_dtypes.bfloat16

B, T, D = 2048, 200, 64
H1, H2 = 80, 40
NCORES = 8
BC = B // NCORES          # 256 batches per core
PAIRS = BC // 2           # 128
NG = PAIRS // 8           # 16 groups of 8 pairs (16 batches)
NCH = PAIRS // 2          # 64 chunks of 2 pairs
GB_CONST = 16             # batches per group


def _build_bass():
    from concourse import bass, bacc, tile
    from concourse import mybir

    dt = mybir.dt
    alu = mybir.AluOpType
    act = mybir.ActivationFunctionType
    nc = bacc.Bacc("TRN2", target_bir_lowering=False, debug=False)

    KTW = GB_CONST * (T + H1)  # 4480 data cols per group slice
    KTP = KTW + 48             # + zero tail so batch 15's lhsT can read 128 cols
    ktwb = nc.declare_dram_parameter("ktwb", [128, NG, KTP], dt.bfloat16, False)
    v2d = nc.declare_dram_parameter("v2d", [128, PAIRS, 2, 128], dt.bfloat16, False)
    amask = nc.declare_dram_parameter("amask", [NG, 128, 2, T], dt.bfloat16, False)
    w2 = nc.declare_dram_parameter("w2", [128, 64], dt.bfloat16, False)
    wfbd = nc.declare_dram_parameter("wfbd", [128, 32], dt.bfloat16, False)
    b2s = nc.declare_dram_parameter("b2s", [128, 1], dt.float32, False)
    ident = nc.declare_dram_parameter("ident", [128, 128], dt.bfloat16, False)
    outp = nc.declare_dram_parameter("outp", [128, PAIRS * 2], dt.float32, True)
    osum = nc.declare_dram_parameter("osum", [128, NG * 2], dt.float32, True)

    GB = GB_CONST   # batches per group
    W2T = 2 * T  # 400

    with tile.TileContext(nc) as tc:
        with (
            tc.tile_pool(name="consts", bufs=1) as cpool,
            tc.tile_pool(name="kin", bufs=2) as kpool,
            tc.tile_pool(name="vin", bufs=3) as vpool,
            tc.tile_pool(name="min", bufs=3) as mpool,
            tc.tile_pool(name="h1", bufs=4) as h1pool,
            tc.tile_pool(name="h2", bufs=3) as h2pool,
            tc.tile_pool(name="ex", bufs=2) as expool,
            tc.tile_pool(name="exm", bufs=2) as exmpool,
            tc.tile_pool(name="ats", bufs=2) as atspool,
            tc.tile_pool(name="ph1", bufs=3, space="PSUM") as ph1pool,
            tc.tile_pool(name="ph2", bufs=2, space="PSUM") as ph2pool,
            tc.tile_pool(name="plg", bufs=2, space="PSUM") as plgpool,
            tc.tile_pool(name="pt", bufs=1, space="PSUM") as ptpool,
        ):
            # kt tiles are [128, GB*280+48]: rows 65-127 and the 48-col tail
            # arrive as zeros from DRAM so every matmul contracts over K=128
            # (HAM never unthrottles the PE clock to 2.4GHz for K<128; the
            # zero rows make padding free — DMA queue time is per-partition
            # bytes). L1's stationary reads 128 cols (wb cols 80-127 are
            # next-batch garbage) — harmless since W2's rows 80-127 are zero.
            # First group's kt/wb lands first (in per-chunk slices) so the PE
            # can start ~1.5us in instead of waiting for the full group.
            # zero warm-up operand (no DMA dependency — the warm-up matmuls
            # below must issue within the first ~0.3us); first group's kt/wb
            # slices split across both HWDGE queues (sync + scalar) so the
            # first chunks' data lands ~2x sooner.
            wz_t = cpool.tile([128, 128], dt.bfloat16, name="wz_t")
            nc.vector.memset(wz_t[:], 0.0)
            kt0 = kpool.tile([128, KTP], dt.bfloat16, name="kt0")
            for jj in range(4):
                hi = KTP if jj == 3 else 1120 * (jj + 1)
                eng = nc.sync if jj % 2 == 0 else nc.scalar
                eng.dma_start(
                    kt0[:, 1120 * jj : hi],
                    ktwb[:, 0, 1120 * jj : hi],
                )

            id_t = cpool.tile([128, 128], dt.bfloat16, name="id_t")
            nc.sync.dma_start(id_t[:], ident[:])
            w2_t = cpool.tile([128, 64], dt.bfloat16, name="w2_t")
            nc.scalar.dma_start(w2_t[:], w2[:])
            wfbd_t = cpool.tile([128, 32], dt.bfloat16, name="wfbd_t")
            nc.scalar.dma_start(wfbd_t[:], wfbd[:])
            b2s_t = cpool.tile([128, 1], dt.float32, name="b2s_t")
            nc.scalar.dma_start(b2s_t[:], b2s[:])

            # Dummy full-K matmuls fill the PE while the first kt DMAs are in
            # flight and trip the HAM busy window, so real work starts at
            # 2.4GHz instead of spending the first ~30us throttled to 1.2.
            warm = plgpool.tile(
                [128, W2T + 16], dt.float32, name="warm", tag="plg"
            )

            def emit_warm(n):
                for _ in range(n):
                    nc.tensor.matmul(
                        warm[:, 0:128],
                        lhsT=wz_t[:],
                        rhs=wz_t[:],
                        start=True,
                        stop=True,
                        skip_group_check=True,
                    )

            emit_warm(24)
            # Warm the DVE vector clock past the const DMAs: TensorScalarPtr
            # (h2-relu with AP scalar) only has one sync-wait slot, so it must
            # not be the first DVE op to observe the b2s DMA completion.
            dve_warm = cpool.tile([128, 1], dt.float32, name="dve_warm")
            nc.vector.tensor_copy(dve_warm[:], b2s_t[:])

            # whole-run accumulators, flushed by one DMA each at the end
            outall = cpool.tile([128, PAIRS * 2], dt.float32, name="outall")
            sum2 = cpool.tile([128, NG * 2], dt.float32, name="sum2")

            kt_t, v2_t, am_t = {}, {}, {}
            ph1_t, h1s_t, ph2_t, h2s_t = {}, {}, {}, {}
            plg_t, exm_t = {}, {}

            def emit_dma(g):
                if g == 0:
                    kt_t[0] = kt0
                else:
                    kt_t[g] = kpool.tile([128, KTP], dt.bfloat16, name="ktg")
                    nc.sync.dma_start(kt_t[g][:], ktwb[:, g, :])
                v2_t[g] = vpool.tile([128, 8, 2, 128], dt.bfloat16, name="v2g")
                nc.sync.dma_start(v2_t[g][:], v2d[:, g * 8 : (g + 1) * 8, :, :])
                am_t[g] = mpool.tile([128, 2, T], dt.bfloat16, name="amg")
                nc.sync.dma_start(am_t[g][:], amask[g])

            def emit_l1_mm(c):
                g, jj = c // 4, c % 4
                KT = kt_t[g]
                for i in range(2):
                    PH1 = ph1pool.tile([128, W2T], dt.float32, name="ph1")
                    ph1_t[(c, i)] = PH1
                    for jb in range(2):
                        bi = 4 * jj + 2 * i + jb
                        o = bi * (T + H1)
                        nc.tensor.matmul(
                            PH1[:, jb * T : (jb + 1) * T],
                            lhsT=KT[:, o + T : o + T + 128],
                            rhs=KT[:, o : o + T],
                            start=True,
                            stop=True,
                            skip_group_check=True,
                        )

            def emit_l1_relu(c):
                for i in range(2):
                    H1S = h1pool.tile([128, W2T], dt.bfloat16, name="h1s")
                    h1s_t[(c, i)] = H1S
                    if i == 0:
                        nc.vector.tensor_scalar_max(
                            H1S[:], ph1_t[(c, i)][:], 0.0
                        )
                    else:
                        nc.scalar.activation(
                            H1S[:], ph1_t[(c, i)][:], act.Relu
                        )

            def emit_l2_mm(cc):
                PH2 = ph2pool.tile([128, W2T], dt.float32, name="ph2")
                ph2_t[cc] = PH2
                for i in range(2):
                    nc.tensor.matmul(
                        PH2[64 * i : 64 * i + 64, :],
                        lhsT=w2_t[:],
                        rhs=h1s_t[(cc, i)][:],
                        start=True,
                        stop=True,
                        tile_position=(0, 64 * i),
                        skip_group_check=True,
                    )

            def emit_l2_h2(cc):
                # h2 feeds L3 on the PE's critical path: it must sit ahead of
                # the bulkier h1 relus in the DVE/ACT queues.
                H2S = h2pool.tile([128, W2T], dt.bfloat16, name="h2s")
                h2s_t[cc] = H2S
                PH2 = ph2_t[cc]
                if cc % 2 == 0:
                    nc.scalar.activation(H2S[:], PH2[:], act.Relu, bias=b2s_t[:])
                else:
                    nc.vector.tensor_scalar(
                        H2S[:], PH2[:], b2s_t[:], 0.0, alu.add, alu.max
                    )

            def emit_l3(cc):
                g, jj = cc // 4, cc % 4
                if jj == 0:
                    plg_t[g] = plgpool.tile(
                        [128, W2T + 16], dt.float32, name="plg", tag="plg"
                    )
                nc.tensor.matmul(
                    plg_t[g][32 * jj : 32 * jj + 32, 0:W2T],
                    lhsT=wfbd_t[:],
                    rhs=h2s_t[cc][:],
                    start=True,
                    stop=True,
                    tile_position=(0, 32 * jj),
                )

            def emit_softmax(g):
                # logits are tiny so exp never overflows; mask applied
                # multiplicatively afterwards with a fused row-sum.
                EX = expool.tile([128, 2, T], dt.bfloat16, name="ex")
                nc.scalar.activation(EX[:], plg_t[g][:, 0:W2T], act.Exp)
                EXM = exmpool.tile([128, 2, 256], dt.bfloat16, name="exm")
                exm_t[g] = EXM
                # zero pad cols so the transposed weights rows 72-127 stay
                # zero (full-K out matmuls); gpsimd is otherwise idle
                nc.gpsimd.memset(EXM[:, :, T:256], 0.0)
                nc.gpsimd.tensor_tensor(
                    out=EXM[:, :, 0:T], in0=EX[:], in1=am_t[g][:], op=alu.mult
                )
                nc.vector.tensor_reduce(
                    out=sum2[:, 2 * g : 2 * g + 2],
                    in_=EXM[:, :, 0:T],
                    axis=mybir.AxisListType.X,
                    op=alu.add,
                )

            ats_t = {}

            def emit_tail_a(g):
                EXM = exm_t[g]
                PT = ptpool.tile([128, 512], dt.bfloat16, name="pt")
                nc.tensor.transpose(PT[0:128, 0:128], EXM[:, 0, 0:128], id_t[:])
                nc.tensor.transpose(PT[0:128, 128:256], EXM[:, 0, 128:256], id_t[:])
                nc.tensor.transpose(PT[0:128, 256:384], EXM[:, 1, 0:128], id_t[:])
                nc.tensor.transpose(PT[0:128, 384:512], EXM[:, 1, 128:256], id_t[:])
                ATS = atspool.tile([128, 2, 256], dt.bfloat16, name="ats")
                ats_t[g] = ATS
                nc.vector.tensor_copy(ATS[:, 0, 0:128], PT[:, 0:128])
                nc.scalar.copy(ATS[:, 0, 128:256], PT[:, 128:256])
                nc.vector.tensor_copy(ATS[:, 1, 0:128], PT[:, 256:384])
                nc.scalar.copy(ATS[:, 1, 128:256], PT[:, 384:512])

            def emit_tail_b(g):
                # out = v^T @ attn^T per pair (v stationary); dst rides in the
                # spare columns of the group's PLG bank.
                ATS = ats_t[g]
                V2 = v2_t[g]
                PLG = plg_t[g]
                for q in range(8):
                    jj, i = q // 2, q % 2
                    ci = 32 * jj + i
                    dst = PLG[:, W2T + 2 * q : W2T + 2 * q + 2]
                    nc.tensor.matmul(
                        dst,
                        lhsT=V2[:, q, 0, :],
                        rhs=ATS[0:128, :, ci],
                        start=True,
                        stop=False,
                    )
                    nc.tensor.matmul(
                        dst,
                        lhsT=V2[:, q, 1, :],
                        rhs=ATS[0:128, :, 128 + ci],
                        start=False,
                        stop=True,
                    )
                nc.vector.tensor_copy(
                    outall[:, 16 * g : 16 * (g + 1)], PLG[:, W2T : W2T + 16]
                )

            emit_dma(0)
            emit_dma(1)
            for c in range(NCH + 8):
                if c < NCH and c % 4 == 0 and c > 0 and c // 4 + 1 < NG:
                    emit_dma(c // 4 + 1)
                if c < 6:
                    emit_warm(6)
                if c < NCH:
                    emit_l1_mm(c)
                if 1 <= c <= NCH:
                    emit_l2_mm(c - 1)
                    emit_l2_h2(c - 1)
                if c < NCH:
                    emit_l1_relu(c)
                if c >= 7 and (c - 7) % 4 == 0 and (c - 7) // 4 < NG:
                    emit_tail_a((c - 7) // 4)
                if c >= 8 and (c - 8) % 4 == 0 and (c - 8) // 4 < NG:
                    emit_tail_b((c - 8) // 4)
                if 3 <= c <= NCH + 2:
                    emit_l3(c - 3)
                if c >= 6 and (c - 6) % 4 == 0 and (c - 6) // 4 < NG:
                    emit_softmax((c - 6) // 4)

            nc.sync.dma_start(outp[:], outall[:])
            nc.sync.dma_start(osum[:], sum2[:])

    nc.compile()
    return nc


_NC_CACHE = {}


def _get_nc():
    if "nc" not in _NC_CACHE:
        _NC_CACHE["nc"] = _build_bass()
    return _NC_CACHE["nc"]


def _prep_core(qc, kc, vc, mc, W1, b1, W2, b2, Wf):
    """Build the per-core DRAM input dict (numpy, host-side)."""
    f32 = np.float32
    W1a, W1b_, W1c, W1d = W1[0:64], W1[64:128], W1[128:192], W1[192:256]

    # ktv [65, BC, 280]: cols 0-199 kt (rows 0-63 = k^T, row 64 = ones),
    # cols 200-279 wb (rows 0-63 = (W1b-W1c) + q_b*W1d, row 64 = beta_b).
    # Shipped zero-padded to 128 rows (+48-col tail) as [128, NG, KTP] so
    # on-device matmuls contract over K=128 (HAM warm) with no memsets.
    ktv = np.empty((D + 1, BC, T + H1), dtype=BF16)
    ktv[0:D, :, 0:T] = kc.transpose(2, 0, 1).astype(BF16)
    ktv[D, :, 0:T] = np.ones((BC, T), dtype=BF16)
    wb_eff = (W1b_ - W1c)[None, :, :] + qc[:, :, None] * W1d[None, :, :]
    beta = qc @ (W1a + W1c) + b1[None, :]
    ktv[0:D, :, T:] = wb_eff.transpose(1, 0, 2).astype(BF16)
    ktv[D, :, T:] = beta.astype(BF16)
    KTW = GB_CONST * (T + H1)
    ktwb = np.zeros((128, NG, KTW + 48), dtype=BF16)
    ktwb[0 : D + 1, :, 0:KTW] = ktv.reshape(D + 1, NG, KTW)

    # v2d [128, PAIRS, 2, 128]: [t%128, pair, t//128, batch-in-pair*64+d]
    vpad = np.zeros((PAIRS, 2, 256, D), dtype=f32)
    vpad[:, :, 0:T] = vc.reshape(PAIRS, 2, T, D)
    v2d = np.ascontiguousarray(
        vpad.reshape(PAIRS, 2, 2, 128, D).transpose(3, 0, 2, 1, 4).reshape(
            128, PAIRS, 2, 128
        )
    ).astype(BF16)

    # amask [NG, 128, 2, T] multiplicative {0,1}, sparse-16 rows {32jj+i}
    m5 = mc.reshape(NG, 4, 2, 2, T)  # [g, jj, i, jb, t]
    am = np.zeros((NG, 128, 2, T), dtype=BF16)
    for jj in range(4):
        for i in range(2):
            am[:, 32 * jj + i, :, :] = m5[:, jj, i].astype(BF16)

    wfbd = np.zeros((128, 32), dtype=BF16)
    wfbd[0:H2, 0] = Wf[:, 0].astype(BF16)
    wfbd[64 : 64 + H2, 1] = Wf[:, 0].astype(BF16)
    b2s = np.zeros((128, 1), dtype=f32)
    b2s[0:H2, 0] = b2
    b2s[64 : 64 + H2, 0] = b2
    w2p = np.zeros((128, 64), dtype=BF16)
    w2p[0:H1, 0:H2] = W2.astype(BF16)

    return {
        "ktwb": ktwb,
        "v2d": v2d,
        "amask": am,
        "w2": w2p,
        "wfbd": wfbd,
        "b2s": b2s,
        "ident": np.eye(128, dtype=BF16),
    }


def _postprocess(res_c):
    """[128,PAIRS*2] unnormalized sums + [128,NG*2] exp-sums -> [BC, D]."""
    op = np.asarray(res_c["outp"], dtype=np.float32).reshape(128, PAIRS, 2)
    osum = np.asarray(res_c["osum"], dtype=np.float32)
    oc = np.empty((BC, D), dtype=np.float32)
    # batch 16g+4jj+2i+jb -> osum[32jj+i, 2g+jb]
    s = np.empty(BC, dtype=np.float32)
    for g in range(NG):
        for jj in range(4):
            for i in range(2):
                for jb in range(2):
                    s[16 * g + 4 * jj + 2 * i + jb] = osum[32 * jj + i, 2 * g + jb]
    s = np.where(s == 0.0, np.float32(1.0), s)
    oc[0::2, :] = op[0:D, :, 0].T / s[0::2][:, None]
    oc[1::2, :] = op[D : 2 * D, :, 1].T / s[1::2][:, None]
    return oc


def kernel(q, k, v, mask, W1, b1, W2, b2, Wf, bf, **_):
    from concourse.bass_utils import run_bass_kernel_spmd

    q = np.asarray(q, dtype=np.float32)
    k = np.asarray(k, dtype=np.float32)
    v = np.asarray(v, dtype=np.float32)
    mask = np.asarray(mask)
    W1 = np.asarray(W1, dtype=np.float32)
    b1 = np.asarray(b1, dtype=np.float32)
    W2 = np.asarray(W2, dtype=np.float32)
    b2 = np.asarray(b2, dtype=np.float32)
    Wf = np.asarray(Wf, dtype=np.float32)

    nc = _get_nc()
    in_maps = []
    for c in range(NCORES):
        s = slice(c * BC, (c + 1) * BC)
        in_maps.append(_prep_core(q[s], k[s], v[s], mask[s], W1, b1, W2, b2, Wf))

    res = run_bass_kernel_spmd(
        nc,
        in_maps,
        list(range(NCORES)),
        tmpdir=os.environ.get("KERNEL_TRACE_DIR") or None,
    )
    globals()["LAST_RES"] = res
    results = res.results

    out = np.empty((B, D), dtype=np.float32)
    for c in range(NCORES):
        out[c * BC : (c + 1) * BC] = _postprocess(results[c])
    return out


if __name__ == "__main__":
    rng = np.random.default_rng(0)
    inputs = {
        "q": rng.standard_normal((B, D), dtype=np.float32),
        "k": rng.standard_normal((B, T, D), dtype=np.float32),
        "v": rng.standard_normal((B, T, D), dtype=np.float32),
        "mask": rng.integers(0, 2, size=(B, T)).astype(np.int32),
        "W1": rng.standard_normal((4 * D, H1), dtype=np.float32) * 0.05,
        "b1": np.zeros(H1, np.float32),
        "W2": rng.standard_normal((H1, H2), dtype=np.float32) * 0.05,
        "b2": np.zeros(H2, np.float32),
        "Wf": rng.standard_normal((H2, 1), dtype=np.float32) * 0.05,
        "bf": np.zeros(1, np.float32),
    }
    out = kernel(**inputs)
    print(out.shape, out.dtype, np.abs(out).max())


# revision 55
# speedup vs baseline: 1.0074x; 1.0074x over previous
"""DIN-style attention layer on 8 Trainium2 NeuronCores.

Problem: q[B,64], k[B,200,64], v[B,200,64], mask[B,200]; per-token MLP on
DIN features concat([q,k,q-k,q*k]) -> 80 -> 40 -> 1 logits, masked softmax
over T, then attn-weighted sum of v. B=2048 sharded over 8 cores.

Math refactor (host):
  info@W1 = q@(W1a+W1c) + k@(W1b-W1c) + (q*k)@W1d   with W1=[W1a;W1b;W1c;W1d]
  => h1_b = relu( Wb_eff^T kt_b + beta_b ),  Wb_eff = (W1b-W1c) + q_b*W1d
     beta_b = q_b@(W1a+W1c) + b1   (folded in as a 65th all-ones row of kt)
bf is dropped: softmax is shift-invariant. mask applied multiplicatively
(exp first, then EXM = EX * mask with fused row-sum on the DVE).

Device (per core, 256 batches = 128 pairs = 64 chunks of 2 pairs):
software-pipelined chunk loop keeping the PE dense (L2 lags one chunk,
L3 two, softmax/transpose/out-matmul tail lags ~a group), elementwise
work balanced across DVE and ACT. Outputs accumulate in SBUF, one DMA
at the end.
"""

import os
import sys

import numpy as np

for _p in ("/opt/trn_rl_repo", "/root/.axon_site/_ro/trn_rl_repo"):
    if os.path.isdir(_p) and _p not in sys.path:
        sys.path.insert(0, _p)

import ml_dtypes

BF16 = ml_dtypes.bfloat16

B, T, D = 2048, 200, 64
H1, H2 = 80, 40
NCORES = 8
BC = B // NCORES          # 256 batches per core
PAIRS = BC // 2           # 128
NG = PAIRS // 8           # 16 groups of 8 pairs (16 batches)
NCH = PAIRS // 2          # 64 chunks of 2 pairs
GB_CONST = 16             # batches per group


def _build_bass():
    from concourse import bass, bacc, tile
    from concourse import mybir

    dt = mybir.dt
    alu = mybir.AluOpType
    act = mybir.ActivationFunctionType
    nc = bacc.Bacc("TRN2", target_bir_lowering=False, debug=False)

    KTW = GB_CONST * (T + H1)  # 4480 data cols per group slice
    KTP = KTW + 48             # + zero tail so batch 15's lhsT can read 128 cols
    ktwb = nc.declare_dram_parameter("ktwb", [128, NG, KTP], dt.bfloat16, False)
    v2d = nc.declare_dram_parameter("v2d", [128, PAIRS, 2, 128], dt.bfloat16, False)
    amask = nc.declare_dram_parameter("amask", [NG, 128, 2, T], dt.bfloat16, False)
    w2 = nc.declare_dram_parameter("w2", [128, 64], dt.bfloat16, False)
    wfbd = nc.declare_dram_parameter("wfbd", [128, 32], dt.bfloat16, False)
    b2s = nc.declare_dram_parameter("b2s", [128, 1], dt.float32, False)
    ident = nc.declare_dram_parameter("ident", [128, 128], dt.bfloat16, False)
    outp = nc.declare_dram_parameter("outp", [128, PAIRS * 2], dt.float32, True)
    osum = nc.declare_dram_parameter("osum", [128, NG * 2], dt.float32, True)

    GB = GB_CONST   # batches per group
    W2T = 2 * T  # 400

    with tile.TileContext(nc) as tc:
        with (
            tc.tile_pool(name="consts", bufs=1) as cpool,
            tc.tile_pool(name="kin", bufs=2) as kpool,
            tc.tile_pool(name="vin", bufs=3) as vpool,
            tc.tile_pool(name="min", bufs=3) as mpool,
            tc.tile_pool(name="h1", bufs=4) as h1pool,
            tc.tile_pool(name="h2", bufs=3) as h2pool,
            tc.tile_pool(name="ex", bufs=2) as expool,
            tc.tile_pool(name="exm", bufs=2) as exmpool,
            tc.tile_pool(name="ats", bufs=2) as atspool,
            tc.tile_pool(name="ph1", bufs=3, space="PSUM") as ph1pool,
            tc.tile_pool(name="ph2", bufs=2, space="PSUM") as ph2pool,
            tc.tile_pool(name="plg", bufs=2, space="PSUM") as plgpool,
            tc.tile_pool(name="pt", bufs=1, space="PSUM") as ptpool,
        ):
            # kt tiles are [128, GB*280+48]: rows 65-127 and the 48-col tail
            # arrive as zeros from DRAM so every matmul contracts over K=128
            # (HAM never unthrottles the PE clock to 2.4GHz for K<128; the
            # zero rows make padding free — DMA queue time is per-partition
            # bytes). L1's stationary reads 128 cols (wb cols 80-127 are
            # next-batch garbage) — harmless since W2's rows 80-127 are zero.
            # First group's kt/wb lands first (in per-chunk slices) so the PE
            # can start ~1.5us in instead of waiting for the full group.
            # zero warm-up operand (no DMA dependency — the warm-up matmuls
            # below must issue within the first ~0.3us); first group's kt/wb
            # slices split across both HWDGE queues (sync + scalar) so the
            # first chunks' data lands ~2x sooner.
            wz_t = cpool.tile([128, 128], dt.bfloat16, name="wz_t")
            nc.vector.memset(wz_t[:], 0.0)
            kt0 = kpool.tile([128, KTP], dt.bfloat16, name="kt0")
            for jj in range(4):
                hi = KTP if jj == 3 else 1120 * (jj + 1)
                eng = nc.sync if jj % 2 == 0 else nc.scalar
                eng.dma_start(
                    kt0[:, 1120 * jj : hi],
                    ktwb[:, 0, 1120 * jj : hi],
                )

            id_t = cpool.tile([128, 128], dt.bfloat16, name="id_t")
            nc.sync.dma_start(id_t[:], ident[:])
            w2_t = cpool.tile([128, 64], dt.bfloat16, name="w2_t")
            nc.scalar.dma_start(w2_t[:], w2[:])
            wfbd_t = cpool.tile([128, 32], dt.bfloat16, name="wfbd_t")
            nc.scalar.dma_start(wfbd_t[:], wfbd[:])
            b2s_t = cpool.tile([128, 1], dt.float32, name="b2s_t")
            nc.scalar.dma_start(b2s_t[:], b2s[:])

            # Dummy full-K matmuls fill the PE while the first kt DMAs are in
            # flight and trip the HAM busy window, so real work starts at
            # 2.4GHz instead of spending the first ~30us throttled to 1.2.
            warm = plgpool.tile(
                [128, W2T + 16], dt.float32, name="warm", tag="plg"
            )

            def emit_warm(n):
                for _ in range(n):
                    nc.tensor.matmul(
                        warm[:, 0:128],
                        lhsT=wz_t[:],
                        rhs=wz_t[:],
                        start=True,
                        stop=True,
                        skip_group_check=True,
                    )

            emit_warm(24)
            # Warm the DVE vector clock past the const DMAs: TensorScalarPtr
            # (h2-relu with AP scalar) only has one sync-wait slot, so it must
            # not be the first DVE op to observe the b2s DMA completion.
            dve_warm = cpool.tile([128, 1], dt.float32, name="dve_warm")
            nc.vector.tensor_copy(dve_warm[:], b2s_t[:])

            # whole-run accumulators, flushed by one DMA each at the end
            outall = cpool.tile([128, PAIRS * 2], dt.float32, name="outall")
            sum2 = cpool.tile([128, NG * 2], dt.float32, name="sum2")

            kt_t, v2_t, am_t = {}, {}, {}
            ph1_t, h1s_t, ph2_t, h2s_t = {}, {}, {}, {}
            plg_t, exm_t = {}, {}

            def emit_dma(g):
                if g == 0:
                    kt_t[0] = kt0
                else:
                    kt_t[g] = kpool.tile([128, KTP], dt.bfloat16, name="ktg")
                    nc.sync.dma_start(kt_t[g][:], ktwb[:, g, :])
                v2_t[g] = vpool.tile([128, 8, 2, 128], dt.bfloat16, name="v2g")
                nc.sync.dma_start(v2_t[g][:], v2d[:, g * 8 : (g + 1) * 8, :, :])
                am_t[g] = mpool.tile([128, 2, T], dt.bfloat16, name="amg")
                nc.sync.dma_start(am_t[g][:], amask[g])

            def emit_l1_mm(c):
                g, jj = c // 4, c % 4
                KT = kt_t[g]
                for i in range(2):
                    PH1 = ph1pool.tile([128, W2T], dt.float32, name="ph1")
                    ph1_t[(c, i)] = PH1
                    for jb in range(2):
                        bi = 4 * jj + 2 * i + jb
                        o = bi * (T + H1)
                        nc.tensor.matmul(
                            PH1[:, jb * T : (jb + 1) * T],
                            lhsT=KT[:, o + T : o + T + 128],
                            rhs=KT[:, o : o + T],
                            start=True,
                            stop=True,
                            skip_group_check=True,
                        )

            def emit_l1_relu(c):
                for i in range(2):
                    H1S = h1pool.tile([128, W2T], dt.bfloat16, name="h1s")
                    h1s_t[(c, i)] = H1S
                    if i == 0:
                        nc.vector.tensor_scalar_max(
                            H1S[:], ph1_t[(c, i)][:], 0.0
                        )
                    else:
                        nc.scalar.activation(
                            H1S[:], ph1_t[(c, i)][:], act.Relu
                        )

            def emit_l2_mm(cc):
                PH2 = ph2pool.tile([128, W2T], dt.float32, name="ph2")
                ph2_t[cc] = PH2
                for i in range(2):
                    nc.tensor.matmul(
                        PH2[64 * i : 64 * i + 64, :],
                        lhsT=w2_t[:],
                        rhs=h1s_t[(cc, i)][:],
                        start=True,
                        stop=True,
                        tile_position=(0, 64 * i),
                        skip_group_check=True,
                    )

            def emit_l2_h2(cc):
                # h2 feeds L3 on the PE's critical path: it must sit ahead of
                # the bulkier h1 relus in the DVE/ACT queues.
                H2S = h2pool.tile([128, W2T], dt.bfloat16, name="h2s")
                h2s_t[cc] = H2S
                PH2 = ph2_t[cc]
                if cc % 2 == 0:
                    nc.scalar.activation(H2S[:], PH2[:], act.Relu, bias=b2s_t[:])
                else:
                    nc.vector.tensor_scalar(
                        H2S[:], PH2[:], b2s_t[:], 0.0, alu.add, alu.max
                    )

            def emit_l3(cc):
                g, jj = cc // 4, cc % 4
                if jj == 0:
                    plg_t[g] = plgpool.tile(
                        [128, W2T + 16], dt.float32, name="plg", tag="plg"
                    )
                nc.tensor.matmul(
                    plg_t[g][32 * jj : 32 * jj + 32, 0:W2T],
                    lhsT=wfbd_t[:],
                    rhs=h2s_t[cc][:],
                    start=True,
                    stop=True,
                    tile_position=(0, 32 * jj),
                )

            def emit_softmax(g):
                # logits are tiny so exp never overflows; mask applied
                # multiplicatively afterwards with a fused row-sum.
                EX = expool.tile([128, 2, T], dt.bfloat16, name="ex")
                nc.scalar.activation(EX[:], plg_t[g][:, 0:W2T], act.Exp)
                EXM = exmpool.tile([128, 2, 256], dt.bfloat16, name="exm")
                exm_t[g] = EXM
                # zero pad cols so the transposed weights rows 72-127 stay
                # zero (full-K out matmuls); gpsimd is otherwise idle
                nc.gpsimd.memset(EXM[:, :, T:256], 0.0)
                nc.vector.tensor_tensor(
                    out=EXM[:, :, 0:T], in0=EX[:], in1=am_t[g][:], op=alu.mult
                )
                nc.vector.tensor_reduce(
                    out=sum2[:, 2 * g : 2 * g + 2],
                    in_=EXM[:, :, 0:T],
                    axis=mybir.AxisListType.X,
                    op=alu.add,
                )

            ats_t = {}

            def emit_tail_a(g):
                EXM = exm_t[g]
                PT = ptpool.tile([128, 512], dt.bfloat16, name="pt")
                nc.tensor.transpose(PT[0:128, 0:128], EXM[:, 0, 0:128], id_t[:])
                nc.tensor.transpose(PT[0:128, 128:256], EXM[:, 0, 128:256], id_t[:])
                nc.tensor.transpose(PT[0:128, 256:384], EXM[:, 1, 0:128], id_t[:])
                nc.tensor.transpose(PT[0:128, 384:512], EXM[:, 1, 128:256], id_t[:])
                ATS = atspool.tile([128, 2, 256], dt.bfloat16, name="ats")
                ats_t[g] = ATS
                nc.vector.tensor_copy(ATS[:, 0, 0:128], PT[:, 0:128])
                nc.scalar.copy(ATS[:, 0, 128:256], PT[:, 128:256])
                nc.vector.tensor_copy(ATS[:, 1, 0:128], PT[:, 256:384])
                nc.scalar.copy(ATS[:, 1, 128:256], PT[:, 384:512])

            def emit_tail_b(g):
                # out = v^T @ attn^T per pair (v stationary); dst rides in the
                # spare columns of the group's PLG bank.
                ATS = ats_t[g]
                V2 = v2_t[g]
                PLG = plg_t[g]
                for q in range(8):
                    jj, i = q // 2, q % 2
                    ci = 32 * jj + i
                    dst = PLG[:, W2T + 2 * q : W2T + 2 * q + 2]
                    nc.tensor.matmul(
                        dst,
                        lhsT=V2[:, q, 0, :],
                        rhs=ATS[0:128, :, ci],
                        start=True,
                        stop=False,
                    )
                    nc.tensor.matmul(
                        dst,
                        lhsT=V2[:, q, 1, :],
                        rhs=ATS[0:128, :, 128 + ci],
                        start=False,
                        stop=True,
                    )
                nc.vector.tensor_copy(
                    outall[:, 16 * g : 16 * (g + 1)], PLG[:, W2T : W2T + 16]
                )

            emit_dma(0)
            emit_dma(1)
            for c in range(NCH + 8):
                if c < NCH and c % 4 == 0 and c > 0 and c // 4 + 1 < NG:
                    emit_dma(c // 4 + 1)
                if c < 6:
                    emit_warm(6)
                if c < NCH:
                    emit_l1_mm(c)
                if 1 <= c <= NCH:
                    emit_l2_mm(c - 1)
                    emit_l2_h2(c - 1)
                if c < NCH:
                    emit_l1_relu(c)
                if c >= 7 and (c - 7) % 4 == 0 and (c - 7) // 4 < NG:
                    emit_tail_a((c - 7) // 4)
                if c >= 8 and (c - 8) % 4 == 0 and (c - 8) // 4 < NG:
                    emit_tail_b((c - 8) // 4)
                if 3 <= c <= NCH + 2:
                    emit_l3(c - 3)
                if c >= 6 and (c - 6) % 4 == 0 and (c - 6) // 4 < NG:
                    emit_softmax((c - 6) // 4)

            nc.sync.dma_start(outp[:], outall[:])
            nc.sync.dma_start(osum[:], sum2[:])

    nc.compile()
    return nc


_NC_CACHE = {}


def _get_nc():
    if "nc" not in _NC_CACHE:
        _NC_CACHE["nc"] = _build_bass()
    return _NC_CACHE["nc"]


def _prep_core(qc, kc, vc, mc, W1, b1, W2, b2, Wf):
    """Build the per-core DRAM input dict (numpy, host-side)."""
    f32 = np.float32
    W1a, W1b_, W1c, W1d = W1[0:64], W1[64:128], W1[128:192], W1[192:256]

    # ktv [65, BC, 280]: cols 0-199 kt (rows 0-63 = k^T, row 64 = ones),
    # cols 200-279 wb (rows 0-63 = (W1b-W1c) + q_b*W1d, row 64 = beta_b).
    # Shipped zero-padded to 128 rows (+48-col tail) as [128, NG, KTP] so
    # on-device matmuls contract over K=128 (HAM warm) with no memsets.
    ktv = np.empty((D + 1, BC, T + H1), dtype=BF16)
    ktv[0:D, :, 0:T] = kc.transpose(2, 0, 1).astype(BF16)
    ktv[D, :, 0:T] = np.ones((BC, T), dtype=BF16)
    wb_eff = (W1b_ - W1c)[None, :, :] + qc[:, :, None] * W1d[None, :, :]
    beta = qc @ (W1a + W1c) + b1[None, :]
    ktv[0:D, :, T:] = wb_eff.transpose(1, 0, 2).astype(BF16)
    ktv[D, :, T:] = beta.astype(BF16)
    KTW = GB_CONST * (T + H1)
    ktwb = np.zeros((128, NG, KTW + 48), dtype=BF16)
    ktwb[0 : D + 1, :, 0:KTW] = ktv.reshape(D + 1, NG, KTW)

    # v2d [128, PAIRS, 2, 128]: [t%128, pair, t//128, batch-in-pair*64+d]
    vpad = np.zeros((PAIRS, 2, 256, D), dtype=f32)
    vpad[:, :, 0:T] = vc.reshape(PAIRS, 2, T, D)
    v2d = np.ascontiguousarray(
        vpad.reshape(PAIRS, 2, 2, 128, D).transpose(3, 0, 2, 1, 4).reshape(
            128, PAIRS, 2, 128
        )
    ).astype(BF16)

    # amask [NG, 128, 2, T] multiplicative {0,1}, sparse-16 rows {32jj+i}
    m5 = mc.reshape(NG, 4, 2, 2, T)  # [g, jj, i, jb, t]
    am = np.zeros((NG, 128, 2, T), dtype=BF16)
    for jj in range(4):
        for i in range(2):
            am[:, 32 * jj + i, :, :] = m5[:, jj, i].astype(BF16)

    wfbd = np.zeros((128, 32), dtype=BF16)
    wfbd[0:H2, 0] = Wf[:, 0].astype(BF16)
    wfbd[64 : 64 + H2, 1] = Wf[:, 0].astype(BF16)
    b2s = np.zeros((128, 1), dtype=f32)
    b2s[0:H2, 0] = b2
    b2s[64 : 64 + H2, 0] = b2
    w2p = np.zeros((128, 64), dtype=BF16)
    w2p[0:H1, 0:H2] = W2.astype(BF16)

    return {
        "ktwb": ktwb,
        "v2d": v2d,
        "amask": am,
        "w2": w2p,
        "wfbd": wfbd,
        "b2s": b2s,
        "ident": np.eye(128, dtype=BF16),
    }


def _postprocess(res_c):
    """[128,PAIRS*2] unnormalized sums + [128,NG*2] exp-sums -> [BC, D]."""
    op = np.asarray(res_c["outp"], dtype=np.float32).reshape(128, PAIRS, 2)
    osum = np.asarray(res_c["osum"], dtype=np.float32)
    oc = np.empty((BC, D), dtype=np.float32)
    # batch 16g+4jj+2i+jb -> osum[32jj+i, 2g+jb]
    s = np.empty(BC, dtype=np.float32)
    for g in range(NG):
        for jj in range(4):
            for i in range(2):
                for jb in range(2):
                    s[16 * g + 4 * jj + 2 * i + jb] = osum[32 * jj + i, 2 * g + jb]
    s = np.where(s == 0.0, np.float32(1.0), s)
    oc[0::2, :] = op[0:D, :, 0].T / s[0::2][:, None]
    oc[1::2, :] = op[D : 2 * D, :, 1].T / s[1::2][:, None]
    return oc


def kernel(q, k, v, mask, W1, b1, W2, b2, Wf, bf, **_):
    from concourse.bass_utils import run_bass_kernel_spmd

    q = np.asarray(q, dtype=np.float32)
    k = np.asarray(k, dtype=np.float32)
    v = np.asarray(v, dtype=np.float32)
    mask = np.asarray(mask)
    W1 = np.asarray(W1, dtype=np.float32)
    b1 = np.asarray(b1, dtype=np.float32)
    W2 = np.asarray(W2, dtype=np.float32)
    b2 = np.asarray(b2, dtype=np.float32)
    Wf = np.asarray(Wf, dtype=np.float32)

    nc = _get_nc()
    in_maps = []
    for c in range(NCORES):
        s = slice(c * BC, (c + 1) * BC)
        in_maps.append(_prep_core(q[s], k[s], v[s], mask[s], W1, b1, W2, b2, Wf))

    res = run_bass_kernel_spmd(
        nc,
        in_maps,
        list(range(NCORES)),
        tmpdir=os.environ.get("KERNEL_TRACE_DIR") or None,
    )
    globals()["LAST_RES"] = res
    results = res.results

    out = np.empty((B, D), dtype=np.float32)
    for c in range(NCORES):
        out[c * BC : (c + 1) * BC] = _postprocess(results[c])
    return out


if __name__ == "__main__":
    rng = np.random.default_rng(0)
    inputs = {
        "q": rng.standard_normal((B, D), dtype=np.float32),
        "k": rng.standard_normal((B, T, D), dtype=np.float32),
        "v": rng.standard_normal((B, T, D), dtype=np.float32),
        "mask": rng.integers(0, 2, size=(B, T)).astype(np.int32),
        "W1": rng.standard_normal((4 * D, H1), dtype=np.float32) * 0.05,
        "b1": np.zeros(H1, np.float32),
        "W2": rng.standard_normal((H1, H2), dtype=np.float32) * 0.05,
        "b2": np.zeros(H2, np.float32),
        "Wf": rng.standard_normal((H2, 1), dtype=np.float32) * 0.05,
        "bf": np.zeros(1, np.float32),
    }
    out = kernel(**inputs)
    print(out.shape, out.dtype, np.abs(out).max())


# revision 56
# speedup vs baseline: 1.0379x; 1.0303x over previous
"""DIN-style attention layer on 8 Trainium2 NeuronCores.

Problem: q[B,64], k[B,200,64], v[B,200,64], mask[B,200]; per-token MLP on
DIN features concat([q,k,q-k,q*k]) -> 80 -> 40 -> 1 logits, masked softmax
over T, then attn-weighted sum of v. B=2048 sharded over 8 cores.

Math refactor (host):
  info@W1 = q@(W1a+W1c) + k@(W1b-W1c) + (q*k)@W1d   with W1=[W1a;W1b;W1c;W1d]
  => h1_b = relu( Wb_eff^T kt_b + beta_b ),  Wb_eff = (W1b-W1c) + q_b*W1d
     beta_b = q_b@(W1a+W1c) + b1   (folded in as a 65th all-ones row of kt)
bf is dropped: softmax is shift-invariant. mask applied multiplicatively
(exp first, then EXM = EX * mask with fused row-sum on the DVE).

Device (per core, 256 batches = 128 pairs = 64 chunks of 2 pairs):
software-pipelined chunk loop keeping the PE dense (L2 lags one chunk,
L3 two, softmax/transpose/out-matmul tail lags ~a group), elementwise
work balanced across DVE and ACT. Outputs accumulate in SBUF, one DMA
at the end.
"""

import os
import sys

import numpy as np

for _p in ("/opt/trn_rl_repo", "/root/.axon_site/_ro/trn_rl_repo"):
    if os.path.isdir(_p) and _p not in sys.path:
        sys.path.insert(0, _p)

import ml_dtypes

BF16 = ml_dtypes.bfloat16

B, T, D = 2048, 200, 64
H1, H2 = 80, 40
NCORES = 8
BC = B // NCORES          # 256 batches per core
PAIRS = BC // 2           # 128
NG = PAIRS // 8           # 16 groups of 8 pairs (16 batches)
NCH = PAIRS // 2          # 64 chunks of 2 pairs
GB_CONST = 16             # batches per group


def _build_bass():
    from concourse import bass, bacc, tile
    from concourse import mybir

    dt = mybir.dt
    alu = mybir.AluOpType
    act = mybir.ActivationFunctionType
    nc = bacc.Bacc("TRN2", target_bir_lowering=False, debug=False)

    KTW = GB_CONST * (T + H1)  # 4480 data cols per group slice
    KTP = KTW + 48             # + zero tail so batch 15's lhsT can read 128 cols
    ktwb = nc.declare_dram_parameter("ktwb", [128, NG, KTP], dt.bfloat16, False)
    v2d = nc.declare_dram_parameter("v2d", [128, PAIRS, 2, 128], dt.bfloat16, False)
    amask = nc.declare_dram_parameter("amask", [NG, 128, 2, T], dt.bfloat16, False)
    w2 = nc.declare_dram_parameter("w2", [128, 64], dt.bfloat16, False)
    wfbd = nc.declare_dram_parameter("wfbd", [128, 32], dt.bfloat16, False)
    b2s = nc.declare_dram_parameter("b2s", [128, 1], dt.float32, False)
    ident = nc.declare_dram_parameter("ident", [128, 128], dt.bfloat16, False)
    outp = nc.declare_dram_parameter("outp", [128, PAIRS * 2], dt.float32, True)
    osum = nc.declare_dram_parameter("osum", [128, NG * 2], dt.float32, True)

    GB = GB_CONST   # batches per group
    W2T = 2 * T  # 400

    with tile.TileContext(nc) as tc:
        with (
            tc.tile_pool(name="consts", bufs=1) as cpool,
            tc.tile_pool(name="kin", bufs=2) as kpool,
            tc.tile_pool(name="vin", bufs=3) as vpool,
            tc.tile_pool(name="min", bufs=3) as mpool,
            tc.tile_pool(name="h1", bufs=4) as h1pool,
            tc.tile_pool(name="h2", bufs=3) as h2pool,
            tc.tile_pool(name="ex", bufs=2) as expool,
            tc.tile_pool(name="exm", bufs=2) as exmpool,
            tc.tile_pool(name="ats", bufs=2) as atspool,
            tc.tile_pool(name="ph1", bufs=3, space="PSUM") as ph1pool,
            tc.tile_pool(name="ph2", bufs=2, space="PSUM") as ph2pool,
            tc.tile_pool(name="plg", bufs=2, space="PSUM") as plgpool,
            tc.tile_pool(name="pt", bufs=1, space="PSUM") as ptpool,
        ):
            # kt tiles are [128, GB*280+48]: rows 65-127 and the 48-col tail
            # arrive as zeros from DRAM so every matmul contracts over K=128
            # (HAM never unthrottles the PE clock to 2.4GHz for K<128; the
            # zero rows make padding free — DMA queue time is per-partition
            # bytes). L1's stationary reads 128 cols (wb cols 80-127 are
            # next-batch garbage) — harmless since W2's rows 80-127 are zero.
            # First group's kt/wb lands first (in per-chunk slices) so the PE
            # can start ~1.5us in instead of waiting for the full group.
            # zero warm-up operand (no DMA dependency — the warm-up matmuls
            # below must issue within the first ~0.3us); first group's kt/wb
            # slices split across both HWDGE queues (sync + scalar) so the
            # first chunks' data lands ~2x sooner.
            wz_t = cpool.tile([128, 128], dt.bfloat16, name="wz_t")
            nc.vector.memset(wz_t[:], 0.0)
            kt0 = kpool.tile([128, KTP], dt.bfloat16, name="kt0")
            for jj in range(4):
                hi = KTP if jj == 3 else 1120 * (jj + 1)
                eng = nc.sync if jj % 2 == 0 else nc.scalar
                eng.dma_start(
                    kt0[:, 1120 * jj : hi],
                    ktwb[:, 0, 1120 * jj : hi],
                )

            id_t = cpool.tile([128, 128], dt.bfloat16, name="id_t")
            nc.sync.dma_start(id_t[:], ident[:])
            w2_t = cpool.tile([128, 64], dt.bfloat16, name="w2_t")
            nc.scalar.dma_start(w2_t[:], w2[:])
            wfbd_t = cpool.tile([128, 32], dt.bfloat16, name="wfbd_t")
            nc.scalar.dma_start(wfbd_t[:], wfbd[:])
            b2s_t = cpool.tile([128, 1], dt.float32, name="b2s_t")
            nc.scalar.dma_start(b2s_t[:], b2s[:])

            # Dummy full-K matmuls fill the PE while the first kt DMAs are in
            # flight and trip the HAM busy window, so real work starts at
            # 2.4GHz instead of spending the first ~30us throttled to 1.2.
            warm = plgpool.tile(
                [128, W2T + 16], dt.float32, name="warm", tag="plg"
            )

            def emit_warm(n):
                for _ in range(n):
                    nc.tensor.matmul(
                        warm[:, 0:128],
                        lhsT=wz_t[:],
                        rhs=wz_t[:],
                        start=True,
                        stop=True,
                        skip_group_check=True,
                    )

            emit_warm(24)
            # Warm the DVE vector clock past the const DMAs: TensorScalarPtr
            # (h2-relu with AP scalar) only has one sync-wait slot, so it must
            # not be the first DVE op to observe the b2s DMA completion.
            dve_warm = cpool.tile([128, 1], dt.float32, name="dve_warm")
            nc.vector.tensor_copy(dve_warm[:], b2s_t[:])

            # whole-run accumulators, flushed by one DMA each at the end
            outall = cpool.tile([128, PAIRS * 2], dt.float32, name="outall")
            sum2 = cpool.tile([128, NG * 2], dt.float32, name="sum2")

            kt_t, v2_t, am_t = {}, {}, {}
            ph1_t, h1s_t, ph2_t, h2s_t = {}, {}, {}, {}
            plg_t, exm_t = {}, {}

            def emit_dma(g):
                if g == 0:
                    kt_t[0] = kt0
                else:
                    kt_t[g] = kpool.tile([128, KTP], dt.bfloat16, name="ktg")
                    nc.sync.dma_start(kt_t[g][:], ktwb[:, g, :])
                v2_t[g] = vpool.tile([128, 8, 2, 128], dt.bfloat16, name="v2g")
                nc.sync.dma_start(v2_t[g][:], v2d[:, g * 8 : (g + 1) * 8, :, :])
                am_t[g] = mpool.tile([128, 2, T], dt.bfloat16, name="amg")
                nc.sync.dma_start(am_t[g][:], amask[g])

            def emit_l1_mm(c):
                g, jj = c // 4, c % 4
                KT = kt_t[g]
                for i in range(2):
                    PH1 = ph1pool.tile([128, W2T], dt.float32, name="ph1")
                    ph1_t[(c, i)] = PH1
                    for jb in range(2):
                        bi = 4 * jj + 2 * i + jb
                        o = bi * (T + H1)
                        nc.tensor.matmul(
                            PH1[:, jb * T : (jb + 1) * T],
                            lhsT=KT[:, o + T : o + T + 128],
                            rhs=KT[:, o : o + T],
                            start=True,
                            stop=True,
                            skip_group_check=True,
                        )

            def emit_l1_relu(c):
                for i in range(2):
                    H1S = h1pool.tile([128, W2T], dt.bfloat16, name="h1s")
                    h1s_t[(c, i)] = H1S
                    if i == 0:
                        nc.vector.tensor_scalar_max(
                            H1S[:], ph1_t[(c, i)][:], 0.0
                        )
                    else:
                        nc.scalar.activation(
                            H1S[:], ph1_t[(c, i)][:], act.Relu
                        )

            def emit_l2_mm(cc):
                PH2 = ph2pool.tile([128, W2T], dt.float32, name="ph2")
                ph2_t[cc] = PH2
                for i in range(2):
                    nc.tensor.matmul(
                        PH2[64 * i : 64 * i + 64, :],
                        lhsT=w2_t[:],
                        rhs=h1s_t[(cc, i)][:],
                        start=True,
                        stop=True,
                        tile_position=(0, 64 * i),
                        skip_group_check=True,
                    )

            def emit_l2_h2(cc):
                # h2 feeds L3 on the PE's critical path: it must sit ahead of
                # the bulkier h1 relus in the DVE/ACT queues.
                H2S = h2pool.tile([128, W2T], dt.bfloat16, name="h2s")
                h2s_t[cc] = H2S
                PH2 = ph2_t[cc]
                if cc % 2 == 0:
                    nc.scalar.activation(H2S[:], PH2[:], act.Relu, bias=b2s_t[:])
                else:
                    nc.vector.tensor_scalar(
                        H2S[:], PH2[:], b2s_t[:], 0.0, alu.add, alu.max
                    )

            def emit_l3(cc):
                g, jj = cc // 4, cc % 4
                if jj == 0:
                    plg_t[g] = plgpool.tile(
                        [128, W2T + 16], dt.float32, name="plg", tag="plg"
                    )
                nc.tensor.matmul(
                    plg_t[g][32 * jj : 32 * jj + 32, 0:W2T],
                    lhsT=wfbd_t[:],
                    rhs=h2s_t[cc][:],
                    start=True,
                    stop=True,
                    tile_position=(0, 32 * jj),
                )

            def emit_softmax(g):
                # logits are tiny so exp never overflows; mask applied
                # multiplicatively afterwards with a fused row-sum.
                EX = expool.tile([128, 2, T], dt.bfloat16, name="ex")
                nc.scalar.activation(EX[:], plg_t[g][:, 0:W2T], act.Exp)
                EXM = exmpool.tile([128, 2, 256], dt.bfloat16, name="exm")
                exm_t[g] = EXM
                # zero pad cols so the transposed weights rows 72-127 stay
                # zero (full-K out matmuls); gpsimd is otherwise idle
                nc.gpsimd.memset(EXM[:, :, T:256], 0.0)
                nc.vector.tensor_tensor(
                    out=EXM[:, :, 0:T], in0=EX[:], in1=am_t[g][:], op=alu.mult
                )
                nc.vector.tensor_reduce(
                    out=sum2[:, 2 * g : 2 * g + 2],
                    in_=EXM[:, :, 0:T],
                    axis=mybir.AxisListType.X,
                    op=alu.add,
                )

            ats_t = {}

            def emit_tail_a(g):
                EXM = exm_t[g]
                PT = ptpool.tile([128, 512], dt.bfloat16, name="pt")
                nc.tensor.transpose(PT[0:128, 0:128], EXM[:, 0, 0:128], id_t[:])
                nc.tensor.transpose(PT[0:128, 128:256], EXM[:, 0, 128:256], id_t[:])
                nc.tensor.transpose(PT[0:128, 256:384], EXM[:, 1, 0:128], id_t[:])
                nc.tensor.transpose(PT[0:128, 384:512], EXM[:, 1, 128:256], id_t[:])
                ATS = atspool.tile([128, 2, 256], dt.bfloat16, name="ats")
                ats_t[g] = ATS
                nc.vector.tensor_copy(ATS[:, 0, 0:128], PT[:, 0:128])
                nc.scalar.copy(ATS[:, 0, 128:256], PT[:, 128:256])
                nc.vector.tensor_copy(ATS[:, 1, 0:128], PT[:, 256:384])
                nc.scalar.copy(ATS[:, 1, 128:256], PT[:, 384:512])

            def emit_tail_b(g):
                # out = v^T @ attn^T per pair (v stationary); dst rides in the
                # spare columns of the group's PLG bank.
                ATS = ats_t[g]
                V2 = v2_t[g]
                PLG = plg_t[g]
                for q in range(8):
                    jj, i = q // 2, q % 2
                    ci = 32 * jj + i
                    dst = PLG[:, W2T + 2 * q : W2T + 2 * q + 2]
                    nc.tensor.matmul(
                        dst,
                        lhsT=V2[:, q, 0, :],
                        rhs=ATS[0:128, :, ci],
                        start=True,
                        stop=False,
                    )
                    nc.tensor.matmul(
                        dst,
                        lhsT=V2[:, q, 1, :],
                        rhs=ATS[0:128, :, 128 + ci],
                        start=False,
                        stop=True,
                    )
                nc.vector.tensor_copy(
                    outall[:, 16 * g : 16 * (g + 1)], PLG[:, W2T : W2T + 16]
                )

            emit_dma(0)
            emit_dma(1)
            for c in range(NCH + 8):
                if c < NCH and c % 4 == 0 and c > 0 and c // 4 + 1 < NG:
                    emit_dma(c // 4 + 1)
                if c < 6:
                    emit_warm(6)
                if c < NCH:
                    emit_l1_mm(c)
                    emit_l1_relu(c)
                if 1 <= c <= NCH:
                    emit_l2_mm(c - 1)
                    emit_l2_h2(c - 1)
                if c >= 7 and (c - 7) % 4 == 0 and (c - 7) // 4 < NG:
                    emit_tail_a((c - 7) // 4)
                if c >= 8 and (c - 8) % 4 == 0 and (c - 8) // 4 < NG:
                    emit_tail_b((c - 8) // 4)
                if 3 <= c <= NCH + 2:
                    emit_l3(c - 3)
                if c >= 6 and (c - 6) % 4 == 0 and (c - 6) // 4 < NG:
                    emit_softmax((c - 6) // 4)

            nc.sync.dma_start(outp[:], outall[:])
            nc.sync.dma_start(osum[:], sum2[:])

    nc.compile()
    return nc


_NC_CACHE = {}


def _get_nc():
    if "nc" not in _NC_CACHE:
        _NC_CACHE["nc"] = _build_bass()
    return _NC_CACHE["nc"]


def _prep_core(qc, kc, vc, mc, W1, b1, W2, b2, Wf):
    """Build the per-core DRAM input dict (numpy, host-side)."""
    f32 = np.float32
    W1a, W1b_, W1c, W1d = W1[0:64], W1[64:128], W1[128:192], W1[192:256]

    # ktv [65, BC, 280]: cols 0-199 kt (rows 0-63 = k^T, row 64 = ones),
    # cols 200-279 wb (rows 0-63 = (W1b-W1c) + q_b*W1d, row 64 = beta_b).
    # Shipped zero-padded to 128 rows (+48-col tail) as [128, NG, KTP] so
    # on-device matmuls contract over K=128 (HAM warm) with no memsets.
    ktv = np.empty((D + 1, BC, T + H1), dtype=BF16)
    ktv[0:D, :, 0:T] = kc.transpose(2, 0, 1).astype(BF16)
    ktv[D, :, 0:T] = np.ones((BC, T), dtype=BF16)
    wb_eff = (W1b_ - W1c)[None, :, :] + qc[:, :, None] * W1d[None, :, :]
    beta = qc @ (W1a + W1c) + b1[None, :]
    ktv[0:D, :, T:] = wb_eff.transpose(1, 0, 2).astype(BF16)
    ktv[D, :, T:] = beta.astype(BF16)
    KTW = GB_CONST * (T + H1)
    ktwb = np.zeros((128, NG, KTW + 48), dtype=BF16)
    ktwb[0 : D + 1, :, 0:KTW] = ktv.reshape(D + 1, NG, KTW)

    # v2d [128, PAIRS, 2, 128]: [t%128, pair, t//128, batch-in-pair*64+d]
    vpad = np.zeros((PAIRS, 2, 256, D), dtype=f32)
    vpad[:, :, 0:T] = vc.reshape(PAIRS, 2, T, D)
    v2d = np.ascontiguousarray(
        vpad.reshape(PAIRS, 2, 2, 128, D).transpose(3, 0, 2, 1, 4).reshape(
            128, PAIRS, 2, 128
        )
    ).astype(BF16)

    # amask [NG, 128, 2, T] multiplicative {0,1}, sparse-16 rows {32jj+i}
    m5 = mc.reshape(NG, 4, 2, 2, T)  # [g, jj, i, jb, t]
    am = np.zeros((NG, 128, 2, T), dtype=BF16)
    for jj in range(4):
        for i in range(2):
            am[:, 32 * jj + i, :, :] = m5[:, jj, i].astype(BF16)

    wfbd = np.zeros((128, 32), dtype=BF16)
    wfbd[0:H2, 0] = Wf[:, 0].astype(BF16)
    wfbd[64 : 64 + H2, 1] = Wf[:, 0].astype(BF16)
    b2s = np.zeros((128, 1), dtype=f32)
    b2s[0:H2, 0] = b2
    b2s[64 : 64 + H2, 0] = b2
    w2p = np.zeros((128, 64), dtype=BF16)
    w2p[0:H1, 0:H2] = W2.astype(BF16)

    return {
        "ktwb": ktwb,
        "v2d": v2d,
        "amask": am,
        "w2": w2p,
        "wfbd": wfbd,
        "b2s": b2s,
        "ident": np.eye(128, dtype=BF16),
    }


def _postprocess(res_c):
    """[128,PAIRS*2] unnormalized sums + [128,NG*2] exp-sums -> [BC, D]."""
    op = np.asarray(res_c["outp"], dtype=np.float32).reshape(128, PAIRS, 2)
    osum = np.asarray(res_c["osum"], dtype=np.float32)
    oc = np.empty((BC, D), dtype=np.float32)
    # batch 16g+4jj+2i+jb -> osum[32jj+i, 2g+jb]
    s = np.empty(BC, dtype=np.float32)
    for g in range(NG):
        for jj in range(4):
            for i in range(2):
                for jb in range(2):
                    s[16 * g + 4 * jj + 2 * i + jb] = osum[32 * jj + i, 2 * g + jb]
    s = np.where(s == 0.0, np.float32(1.0), s)
    oc[0::2, :] = op[0:D, :, 0].T / s[0::2][:, None]
    oc[1::2, :] = op[D : 2 * D, :, 1].T / s[1::2][:, None]
    return oc


def kernel(q, k, v, mask, W1, b1, W2, b2, Wf, bf, **_):
    from concourse.bass_utils import run_bass_kernel_spmd

    q = np.asarray(q, dtype=np.float32)
    k = np.asarray(k, dtype=np.float32)
    v = np.asarray(v, dtype=np.float32)
    mask = np.asarray(mask)
    W1 = np.asarray(W1, dtype=np.float32)
    b1 = np.asarray(b1, dtype=np.float32)
    W2 = np.asarray(W2, dtype=np.float32)
    b2 = np.asarray(b2, dtype=np.float32)
    Wf = np.asarray(Wf, dtype=np.float32)

    nc = _get_nc()
    in_maps = []
    for c in range(NCORES):
        s = slice(c * BC, (c + 1) * BC)
        in_maps.append(_prep_core(q[s], k[s], v[s], mask[s], W1, b1, W2, b2, Wf))

    res = run_bass_kernel_spmd(
        nc,
        in_maps,
        list(range(NCORES)),
        tmpdir=os.environ.get("KERNEL_TRACE_DIR") or None,
    )
    globals()["LAST_RES"] = res
    results = res.results

    out = np.empty((B, D), dtype=np.float32)
    for c in range(NCORES):
        out[c * BC : (c + 1) * BC] = _postprocess(results[c])
    return out


if __name__ == "__main__":
    rng = np.random.default_rng(0)
    inputs = {
        "q": rng.standard_normal((B, D), dtype=np.float32),
        "k": rng.standard_normal((B, T, D), dtype=np.float32),
        "v": rng.standard_normal((B, T, D), dtype=np.float32),
        "mask": rng.integers(0, 2, size=(B, T)).astype(np.int32),
        "W1": rng.standard_normal((4 * D, H1), dtype=np.float32) * 0.05,
        "b1": np.zeros(H1, np.float32),
        "W2": rng.standard_normal((H1, H2), dtype=np.float32) * 0.05,
        "b2": np.zeros(H2, np.float32),
        "Wf": rng.standard_normal((H2, 1), dtype=np.float32) * 0.05,
        "bf": np.zeros(1, np.float32),
    }
    out = kernel(**inputs)
    print(out.shape, out.dtype, np.abs(out).max())


# revision 59
# speedup vs baseline: 1.0621x; 1.0232x over previous
"""DIN-style attention layer on 8 Trainium2 NeuronCores.

Problem: q[B,64], k[B,200,64], v[B,200,64], mask[B,200]; per-token MLP on
DIN features concat([q,k,q-k,q*k]) -> 80 -> 40 -> 1 logits, masked softmax
over T, then attn-weighted sum of v. B=2048 sharded over 8 cores.

Math refactor (host):
  info@W1 = q@(W1a+W1c) + k@(W1b-W1c) + (q*k)@W1d   with W1=[W1a;W1b;W1c;W1d]
  => h1_b = relu( Wb_eff^T kt_b + beta_b ),  Wb_eff = (W1b-W1c) + q_b*W1d
     beta_b = q_b@(W1a+W1c) + b1   (folded in as a 65th all-ones row of kt)
bf is dropped: softmax is shift-invariant. mask applied multiplicatively
(exp first, then EXM = EX * mask with fused row-sum on the DVE).

Device (per core, 256 batches = 128 pairs = 64 chunks of 2 pairs):
software-pipelined chunk loop keeping the PE dense (L2 lags one chunk,
L3 two, softmax/transpose/out-matmul tail lags ~a group), elementwise
work balanced across DVE and ACT. Outputs accumulate in SBUF, one DMA
at the end.
"""

import os
import sys

import numpy as np

for _p in ("/opt/trn_rl_repo", "/root/.axon_site/_ro/trn_rl_repo"):
    if os.path.isdir(_p) and _p not in sys.path:
        sys.path.insert(0, _p)

import ml_dtypes

BF16 = ml_dtypes.bfloat16

B, T, D = 2048, 200, 64
H1, H2 = 80, 40
NCORES = 8
BC = B // NCORES          # 256 batches per core
PAIRS = BC // 2           # 128
NG = PAIRS // 8           # 16 groups of 8 pairs (16 batches)
NCH = PAIRS // 2          # 64 chunks of 2 pairs
GB_CONST = 16             # batches per group


def _build_bass():
    from concourse import bass, bacc, tile
    from concourse import mybir

    dt = mybir.dt
    alu = mybir.AluOpType
    act = mybir.ActivationFunctionType
    nc = bacc.Bacc("TRN2", target_bir_lowering=False, debug=False)

    KTW = GB_CONST * (T + H1)  # 4480 data cols per group slice
    KTP = KTW + 48             # + zero tail so batch 15's lhsT can read 128 cols
    ktwb = nc.declare_dram_parameter("ktwb", [128, NG, KTP], dt.bfloat16, False)
    v2d = nc.declare_dram_parameter("v2d", [128, PAIRS, 2, 128], dt.bfloat16, False)
    amask = nc.declare_dram_parameter("amask", [NG, 128, 2, T], dt.bfloat16, False)
    w2 = nc.declare_dram_parameter("w2", [128, 64], dt.bfloat16, False)
    wfbd = nc.declare_dram_parameter("wfbd", [128, 32], dt.bfloat16, False)
    b2s = nc.declare_dram_parameter("b2s", [128, 1], dt.float32, False)
    ident = nc.declare_dram_parameter("ident", [128, 128], dt.bfloat16, False)
    outp = nc.declare_dram_parameter("outp", [128, PAIRS * 2], dt.float32, True)
    osum = nc.declare_dram_parameter("osum", [128, NG * 2], dt.float32, True)

    GB = GB_CONST   # batches per group
    W2T = 2 * T  # 400

    with tile.TileContext(nc) as tc:
        with (
            tc.tile_pool(name="consts", bufs=1) as cpool,
            tc.tile_pool(name="kin", bufs=2) as kpool,
            tc.tile_pool(name="vin", bufs=3) as vpool,
            tc.tile_pool(name="min", bufs=3) as mpool,
            tc.tile_pool(name="h1", bufs=4) as h1pool,
            tc.tile_pool(name="h2", bufs=4) as h2pool,
            tc.tile_pool(name="ex", bufs=2) as expool,
            tc.tile_pool(name="exm", bufs=2) as exmpool,
            tc.tile_pool(name="ats", bufs=2) as atspool,
            tc.tile_pool(name="ph1", bufs=3, space="PSUM") as ph1pool,
            tc.tile_pool(name="ph2", bufs=2, space="PSUM") as ph2pool,
            tc.tile_pool(name="plg", bufs=2, space="PSUM") as plgpool,
            tc.tile_pool(name="pt", bufs=1, space="PSUM") as ptpool,
        ):
            # kt tiles are [128, GB*280+48]: rows 65-127 and the 48-col tail
            # arrive as zeros from DRAM so every matmul contracts over K=128
            # (HAM never unthrottles the PE clock to 2.4GHz for K<128; the
            # zero rows make padding free — DMA queue time is per-partition
            # bytes). L1's stationary reads 128 cols (wb cols 80-127 are
            # next-batch garbage) — harmless since W2's rows 80-127 are zero.
            # First group's kt/wb lands first (in per-chunk slices) so the PE
            # can start ~1.5us in instead of waiting for the full group.
            # zero warm-up operand (no DMA dependency — the warm-up matmuls
            # below must issue within the first ~0.3us); first group's kt/wb
            # slices split across both HWDGE queues (sync + scalar) so the
            # first chunks' data lands ~2x sooner.
            wz_t = cpool.tile([128, 128], dt.bfloat16, name="wz_t")
            nc.vector.memset(wz_t[:], 0.0)
            kt0 = kpool.tile([128, KTP], dt.bfloat16, name="kt0")
            for jj in range(4):
                hi = KTP if jj == 3 else 1120 * (jj + 1)
                eng = nc.sync if jj % 2 == 0 else nc.scalar
                eng.dma_start(
                    kt0[:, 1120 * jj : hi],
                    ktwb[:, 0, 1120 * jj : hi],
                )

            id_t = cpool.tile([128, 128], dt.bfloat16, name="id_t")
            nc.sync.dma_start(id_t[:], ident[:])
            w2_t = cpool.tile([128, 64], dt.bfloat16, name="w2_t")
            nc.scalar.dma_start(w2_t[:], w2[:])
            wfbd_t = cpool.tile([128, 32], dt.bfloat16, name="wfbd_t")
            nc.scalar.dma_start(wfbd_t[:], wfbd[:])
            b2s_t = cpool.tile([128, 1], dt.float32, name="b2s_t")
            nc.scalar.dma_start(b2s_t[:], b2s[:])

            # Dummy full-K matmuls fill the PE while the first kt DMAs are in
            # flight and trip the HAM busy window, so real work starts at
            # 2.4GHz instead of spending the first ~30us throttled to 1.2.
            warm = plgpool.tile(
                [128, W2T + 16], dt.float32, name="warm", tag="plg"
            )

            def emit_warm(n):
                for _ in range(n):
                    nc.tensor.matmul(
                        warm[:, 0:128],
                        lhsT=wz_t[:],
                        rhs=wz_t[:],
                        start=True,
                        stop=True,
                        skip_group_check=True,
                    )

            emit_warm(24)
            # Warm the DVE vector clock past the const DMAs: TensorScalarPtr
            # (h2-relu with AP scalar) only has one sync-wait slot, so it must
            # not be the first DVE op to observe the b2s DMA completion.
            dve_warm = cpool.tile([128, 1], dt.float32, name="dve_warm")
            nc.vector.tensor_copy(dve_warm[:], b2s_t[:])

            # whole-run accumulators, flushed by one DMA each at the end
            outall = cpool.tile([128, PAIRS * 2], dt.float32, name="outall")
            sum2 = cpool.tile([128, NG * 2], dt.float32, name="sum2")

            kt_t, v2_t, am_t = {}, {}, {}
            ph1_t, h1s_t, ph2_t, h2s_t = {}, {}, {}, {}
            plg_t, exm_t = {}, {}

            def emit_dma(g):
                if g == 0:
                    kt_t[0] = kt0
                else:
                    kt_t[g] = kpool.tile([128, KTP], dt.bfloat16, name="ktg")
                    nc.sync.dma_start(kt_t[g][:], ktwb[:, g, :])
                v2_t[g] = vpool.tile([128, 8, 2, 128], dt.bfloat16, name="v2g")
                nc.sync.dma_start(v2_t[g][:], v2d[:, g * 8 : (g + 1) * 8, :, :])
                am_t[g] = mpool.tile([128, 2, T], dt.bfloat16, name="amg")
                nc.sync.dma_start(am_t[g][:], amask[g])

            def emit_l1_mm(c):
                g, jj = c // 4, c % 4
                KT = kt_t[g]
                for i in range(2):
                    PH1 = ph1pool.tile([128, W2T], dt.float32, name="ph1")
                    ph1_t[(c, i)] = PH1
                    for jb in range(2):
                        bi = 4 * jj + 2 * i + jb
                        o = bi * (T + H1)
                        nc.tensor.matmul(
                            PH1[:, jb * T : (jb + 1) * T],
                            lhsT=KT[:, o + T : o + T + 128],
                            rhs=KT[:, o : o + T],
                            start=True,
                            stop=True,
                            skip_group_check=True,
                        )

            def emit_l1_relu(c):
                for i in range(2):
                    H1S = h1pool.tile([128, W2T], dt.bfloat16, name="h1s")
                    h1s_t[(c, i)] = H1S
                    if i == 0:
                        nc.vector.tensor_scalar_max(
                            H1S[:], ph1_t[(c, i)][:], 0.0
                        )
                    else:
                        nc.scalar.activation(
                            H1S[:], ph1_t[(c, i)][:], act.Relu
                        )

            def emit_l2_mm(cc):
                PH2 = ph2pool.tile([128, W2T], dt.float32, name="ph2")
                ph2_t[cc] = PH2
                for i in range(2):
                    nc.tensor.matmul(
                        PH2[64 * i : 64 * i + 64, :],
                        lhsT=w2_t[:],
                        rhs=h1s_t[(cc, i)][:],
                        start=True,
                        stop=True,
                        tile_position=(0, 64 * i),
                        skip_group_check=True,
                    )

            def emit_l2_h2(cc):
                # h2 feeds L3 on the PE's critical path: it must sit ahead of
                # the bulkier h1 relus in the DVE/ACT queues.
                H2S = h2pool.tile([128, W2T], dt.bfloat16, name="h2s")
                h2s_t[cc] = H2S
                PH2 = ph2_t[cc]
                if cc % 2 == 0:
                    nc.scalar.activation(H2S[:], PH2[:], act.Relu, bias=b2s_t[:])
                else:
                    nc.vector.tensor_scalar(
                        H2S[:], PH2[:], b2s_t[:], 0.0, alu.add, alu.max
                    )

            def emit_l3(cc):
                g, jj = cc // 4, cc % 4
                if jj == 0:
                    plg_t[g] = plgpool.tile(
                        [128, W2T + 16], dt.float32, name="plg", tag="plg"
                    )
                nc.tensor.matmul(
                    plg_t[g][32 * jj : 32 * jj + 32, 0:W2T],
                    lhsT=wfbd_t[:],
                    rhs=h2s_t[cc][:],
                    start=True,
                    stop=True,
                    tile_position=(0, 32 * jj),
                )

            def emit_softmax(g):
                # logits are tiny so exp never overflows; mask applied
                # multiplicatively afterwards with a fused row-sum.
                EX = expool.tile([128, 2, T], dt.bfloat16, name="ex")
                nc.scalar.activation(EX[:], plg_t[g][:, 0:W2T], act.Exp)
                EXM = exmpool.tile([128, 2, 256], dt.bfloat16, name="exm")
                exm_t[g] = EXM
                # zero pad cols so the transposed weights rows 72-127 stay
                # zero (full-K out matmuls); gpsimd is otherwise idle
                nc.gpsimd.memset(EXM[:, :, T:256], 0.0)
                nc.vector.tensor_tensor(
                    out=EXM[:, :, 0:T], in0=EX[:], in1=am_t[g][:], op=alu.mult
                )
                nc.vector.tensor_reduce(
                    out=sum2[:, 2 * g : 2 * g + 2],
                    in_=EXM[:, :, 0:T],
                    axis=mybir.AxisListType.X,
                    op=alu.add,
                )

            ats_t = {}

            def emit_tail_a(g):
                EXM = exm_t[g]
                PT = ptpool.tile([128, 512], dt.bfloat16, name="pt")
                nc.tensor.transpose(PT[0:128, 0:128], EXM[:, 0, 0:128], id_t[:])
                nc.tensor.transpose(PT[0:128, 128:256], EXM[:, 0, 128:256], id_t[:])
                nc.tensor.transpose(PT[0:128, 256:384], EXM[:, 1, 0:128], id_t[:])
                nc.tensor.transpose(PT[0:128, 384:512], EXM[:, 1, 128:256], id_t[:])
                ATS = atspool.tile([128, 2, 256], dt.bfloat16, name="ats")
                ats_t[g] = ATS
                nc.vector.tensor_copy(ATS[:, 0, 0:128], PT[:, 0:128])
                nc.scalar.copy(ATS[:, 0, 128:256], PT[:, 128:256])
                nc.vector.tensor_copy(ATS[:, 1, 0:128], PT[:, 256:384])
                nc.scalar.copy(ATS[:, 1, 128:256], PT[:, 384:512])

            def emit_tail_b(g):
                # out = v^T @ attn^T per pair (v stationary); dst rides in the
                # spare columns of the group's PLG bank.
                ATS = ats_t[g]
                V2 = v2_t[g]
                PLG = plg_t[g]
                for q in range(8):
                    jj, i = q // 2, q % 2
                    ci = 32 * jj + i
                    dst = PLG[:, W2T + 2 * q : W2T + 2 * q + 2]
                    nc.tensor.matmul(
                        dst,
                        lhsT=V2[:, q, 0, :],
                        rhs=ATS[0:128, :, ci],
                        start=True,
                        stop=False,
                    )
                    nc.tensor.matmul(
                        dst,
                        lhsT=V2[:, q, 1, :],
                        rhs=ATS[0:128, :, 128 + ci],
                        start=False,
                        stop=True,
                    )
                nc.vector.tensor_copy(
                    outall[:, 16 * g : 16 * (g + 1)], PLG[:, W2T : W2T + 16]
                )

            emit_dma(0)
            emit_dma(1)
            for c in range(NCH + 12):
                if c < NCH and c % 4 == 0 and c > 0 and c // 4 + 1 < NG:
                    emit_dma(c // 4 + 1)
                if c < 6:
                    emit_warm(6)
                if c < NCH:
                    emit_l1_mm(c)
                    emit_l1_relu(c)
                if 1 <= c <= NCH:
                    emit_l2_mm(c - 1)
                    emit_l2_h2(c - 1)
                if c >= 8 and (c - 8) % 4 == 0 and (c - 8) // 4 < NG:
                    emit_tail_a((c - 8) // 4)
                if c >= 9 and (c - 9) % 4 == 0 and (c - 9) // 4 < NG:
                    emit_tail_b((c - 9) // 4)
                # the group's last L3 gets one extra chunk of lag: its h2
                # lands late in the DVE/ACT queues and the PE would stall
                if 3 <= c <= NCH + 2 and (c - 3) % 4 != 3:
                    emit_l3(c - 3)
                if 4 <= c <= NCH + 3 and (c - 4) % 4 == 3:
                    emit_l3(c - 4)
                if c >= 7 and (c - 7) % 4 == 0 and (c - 7) // 4 < NG:
                    emit_softmax((c - 7) // 4)

            nc.sync.dma_start(outp[:], outall[:])
            nc.sync.dma_start(osum[:], sum2[:])

    nc.compile()
    return nc


_NC_CACHE = {}


def _get_nc():
    if "nc" not in _NC_CACHE:
        _NC_CACHE["nc"] = _build_bass()
    return _NC_CACHE["nc"]


def _prep_core(qc, kc, vc, mc, W1, b1, W2, b2, Wf):
    """Build the per-core DRAM input dict (numpy, host-side)."""
    f32 = np.float32
    W1a, W1b_, W1c, W1d = W1[0:64], W1[64:128], W1[128:192], W1[192:256]

    # ktv [65, BC, 280]: cols 0-199 kt (rows 0-63 = k^T, row 64 = ones),
    # cols 200-279 wb (rows 0-63 = (W1b-W1c) + q_b*W1d, row 64 = beta_b).
    # Shipped zero-padded to 128 rows (+48-col tail) as [128, NG, KTP] so
    # on-device matmuls contract over K=128 (HAM warm) with no memsets.
    ktv = np.empty((D + 1, BC, T + H1), dtype=BF16)
    ktv[0:D, :, 0:T] = kc.transpose(2, 0, 1).astype(BF16)
    ktv[D, :, 0:T] = np.ones((BC, T), dtype=BF16)
    wb_eff = (W1b_ - W1c)[None, :, :] + qc[:, :, None] * W1d[None, :, :]
    beta = qc @ (W1a + W1c) + b1[None, :]
    ktv[0:D, :, T:] = wb_eff.transpose(1, 0, 2).astype(BF16)
    ktv[D, :, T:] = beta.astype(BF16)
    KTW = GB_CONST * (T + H1)
    ktwb = np.zeros((128, NG, KTW + 48), dtype=BF16)
    ktwb[0 : D + 1, :, 0:KTW] = ktv.reshape(D + 1, NG, KTW)

    # v2d [128, PAIRS, 2, 128]: [t%128, pair, t//128, batch-in-pair*64+d]
    vpad = np.zeros((PAIRS, 2, 256, D), dtype=f32)
    vpad[:, :, 0:T] = vc.reshape(PAIRS, 2, T, D)
    v2d = np.ascontiguousarray(
        vpad.reshape(PAIRS, 2, 2, 128, D).transpose(3, 0, 2, 1, 4).reshape(
            128, PAIRS, 2, 128
        )
    ).astype(BF16)

    # amask [NG, 128, 2, T] multiplicative {0,1}, sparse-16 rows {32jj+i}
    m5 = mc.reshape(NG, 4, 2, 2, T)  # [g, jj, i, jb, t]
    am = np.zeros((NG, 128, 2, T), dtype=BF16)
    for jj in range(4):
        for i in range(2):
            am[:, 32 * jj + i, :, :] = m5[:, jj, i].astype(BF16)

    wfbd = np.zeros((128, 32), dtype=BF16)
    wfbd[0:H2, 0] = Wf[:, 0].astype(BF16)
    wfbd[64 : 64 + H2, 1] = Wf[:, 0].astype(BF16)
    b2s = np.zeros((128, 1), dtype=f32)
    b2s[0:H2, 0] = b2
    b2s[64 : 64 + H2, 0] = b2
    w2p = np.zeros((128, 64), dtype=BF16)
    w2p[0:H1, 0:H2] = W2.astype(BF16)

    return {
        "ktwb": ktwb,
        "v2d": v2d,
        "amask": am,
        "w2": w2p,
        "wfbd": wfbd,
        "b2s": b2s,
        "ident": np.eye(128, dtype=BF16),
    }


def _postprocess(res_c):
    """[128,PAIRS*2] unnormalized sums + [128,NG*2] exp-sums -> [BC, D]."""
    op = np.asarray(res_c["outp"], dtype=np.float32).reshape(128, PAIRS, 2)
    osum = np.asarray(res_c["osum"], dtype=np.float32)
    oc = np.empty((BC, D), dtype=np.float32)
    # batch 16g+4jj+2i+jb -> osum[32jj+i, 2g+jb]
    s = np.empty(BC, dtype=np.float32)
    for g in range(NG):
        for jj in range(4):
            for i in range(2):
                for jb in range(2):
                    s[16 * g + 4 * jj + 2 * i + jb] = osum[32 * jj + i, 2 * g + jb]
    s = np.where(s == 0.0, np.float32(1.0), s)
    oc[0::2, :] = op[0:D, :, 0].T / s[0::2][:, None]
    oc[1::2, :] = op[D : 2 * D, :, 1].T / s[1::2][:, None]
    return oc


def kernel(q, k, v, mask, W1, b1, W2, b2, Wf, bf, **_):
    from concourse.bass_utils import run_bass_kernel_spmd

    q = np.asarray(q, dtype=np.float32)
    k = np.asarray(k, dtype=np.float32)
    v = np.asarray(v, dtype=np.float32)
    mask = np.asarray(mask)
    W1 = np.asarray(W1, dtype=np.float32)
    b1 = np.asarray(b1, dtype=np.float32)
    W2 = np.asarray(W2, dtype=np.float32)
    b2 = np.asarray(b2, dtype=np.float32)
    Wf = np.asarray(Wf, dtype=np.float32)

    nc = _get_nc()
    in_maps = []
    for c in range(NCORES):
        s = slice(c * BC, (c + 1) * BC)
        in_maps.append(_prep_core(q[s], k[s], v[s], mask[s], W1, b1, W2, b2, Wf))

    res = run_bass_kernel_spmd(
        nc,
        in_maps,
        list(range(NCORES)),
        tmpdir=os.environ.get("KERNEL_TRACE_DIR") or None,
    )
    globals()["LAST_RES"] = res
    results = res.results

    out = np.empty((B, D), dtype=np.float32)
    for c in range(NCORES):
        out[c * BC : (c + 1) * BC] = _postprocess(results[c])
    return out


if __name__ == "__main__":
    rng = np.random.default_rng(0)
    inputs = {
        "q": rng.standard_normal((B, D), dtype=np.float32),
        "k": rng.standard_normal((B, T, D), dtype=np.float32),
        "v": rng.standard_normal((B, T, D), dtype=np.float32),
        "mask": rng.integers(0, 2, size=(B, T)).astype(np.int32),
        "W1": rng.standard_normal((4 * D, H1), dtype=np.float32) * 0.05,
        "b1": np.zeros(H1, np.float32),
        "W2": rng.standard_normal((H1, H2), dtype=np.float32) * 0.05,
        "b2": np.zeros(H2, np.float32),
        "Wf": rng.standard_normal((H2, 1), dtype=np.float32) * 0.05,
        "bf": np.zeros(1, np.float32),
    }
    out = kernel(**inputs)
    print(out.shape, out.dtype, np.abs(out).max())
